# revision 3
# baseline (speedup 1.0000x reference)
"""Trainium2 Bass kernel for nn_AsymmetricLossCustom (8 NeuronCores).

Math (reference):
    s  = sigmoid(x)
    t  = min(1 - s + 0.05, 1)
    loss = y*ln(max(s,eps)) + (1-y)*ln(max(t,eps))        # [B, C]
    active[b,c] = OR_g ( (any_g[b] & ~has_g[b]) & mask_g[c] )
    out = -(loss * where(active, 0.1, 1.0)).sum()

Device-friendly rewrite (single ACT table set: natural_log_exp):
    u   = exp(-x)
    L1  = ln(1 + u)          = -ln(s)
    L05 = ln(0.05 + 1.05*u)  = ln(1.05 - s) + L1
    m   = min(L05, L1)       # = ln(t) + L1
    loss = -L1 + (1-y)*m
    sum(loss) = -sum(L1) - sum((y-1)*m)     # both accumulate via fused accum_out

The `active` down-weighting only touches columns appearing in one of the
three index arrays (<=170 of 9605), so it is computed as a correction term
over host-gathered columns:
    out = -(sum(loss)) + 0.9 * sum(loss * active)
        = main_neg - 0.9 * corr_neg
with main_neg = sum(L1) + sum((y-1)*m), corr_neg = sum(active * (-loss)).

Sharding: pure data parallel over the batch. Each core gets 512 rows,
viewed as [128 partitions, 38420 free] (4 rows per partition, contiguous),
plus the gathered columns [512, 176] and replicated [128, 176] group masks.
Host sums the 8 per-core [128, 2] partial outputs.
"""

import sys

import numpy as np

if "/opt/trn_rl_repo" not in sys.path:
    sys.path.insert(0, "/opt/trn_rl_repo")

B, C = 4096, 9605
NCORES = 8
ROWS = B // NCORES          # 512 rows per core
P = 128                     # SBUF partitions
FREE = (ROWS // P) * C      # 38420 f32 per partition
NCHUNK = 17
F = FREE // NCHUNK          # 2260
NBT = ROWS // P             # 4 gathered batch-tiles per core
U_PAD = 176                 # padded union-column count (>= 70+70+30)
CLIP = 0.05
ALPHA = 0.1

TRACE = False               # set True (e.g. from test.py) to capture an NTFF profile
LAST_RESULTS = None         # BassKernelResults of the most recent run

_NC = None


def _build_program():
    import concourse.bacc as bacc
    import concourse.mybir as mybir
    from concourse import tile

    f32 = mybir.dt.float32
    Alu = mybir.AluOpType
    Act = mybir.ActivationFunctionType
    AX = mybir.AxisListType

    nc = bacc.Bacc(
        "TRN2",
        target_bir_lowering=False,
        debug=False,
        enable_asserts=False,
        num_devices=NCORES,
    )

    x = nc.dram_tensor("x", [P, FREE], f32, kind="ExternalInput").ap()
    y = nc.dram_tensor("y", [P, FREE], f32, kind="ExternalInput").ap()
    xg = nc.dram_tensor("xg", [ROWS, U_PAD], f32, kind="ExternalInput").ap()
    yg = nc.dram_tensor("yg", [ROWS, U_PAD], f32, kind="ExternalInput").ap()
    mr = nc.dram_tensor("mr", [P, U_PAD], f32, kind="ExternalInput").ap()
    md = nc.dram_tensor("md", [P, U_PAD], f32, kind="ExternalInput").ap()
    mc = nc.dram_tensor("mc", [P, U_PAD], f32, kind="ExternalInput").ap()
    out = nc.dram_tensor("out", [P, 2], f32, kind="ExternalOutput").ap()

    with tile.TileContext(nc) as tc:
        with (
            tc.tile_pool(name="xp", bufs=4) as xp,
            tc.tile_pool(name="yp", bufs=4) as yp,
            tc.tile_pool(name="up", bufs=2) as up,
            tc.tile_pool(name="l1p", bufs=2) as l1p,
            tc.tile_pool(name="l5p", bufs=2) as l5p,
            tc.tile_pool(name="accp", bufs=1) as accp,
            tc.tile_pool(name="gp", bufs=2) as gp,
            tc.tile_pool(name="maskp", bufs=1) as maskp,
            tc.tile_pool(name="junkp", bufs=3) as junkp,
            tc.tile_pool(name="flagp", bufs=2) as flagp,
            tc.tile_pool(name="finp", bufs=1) as finp,
        ):
            accL1 = accp.tile([P, NCHUNK], f32, tag="accL1")
            accM = accp.tile([P, NCHUNK], f32, tag="accM")
            accC = accp.tile([P, NBT], f32, tag="accC")

            # 0.05 bias for the Ln(1.05*u + 0.05) pass (bias must be an AP)
            bclip = accp.tile([P, 1], f32, tag="bclip")
            nc.vector.memset(bclip[:], CLIP)

            # ---------------- main stream: sum(loss) over the full shard ----
            for k in range(NCHUNK):
                cs = slice(k * F, (k + 1) * F)
                xt = xp.tile([P, F], f32, tag="x")
                nc.sync.dma_start(xt[:], x[:, cs])
                yt = yp.tile([P, F], f32, tag="y")
                nc.sync.dma_start(yt[:], y[:, cs])

                ut = up.tile([P, F], f32, tag="u")
                nc.scalar.activation(ut[:], xt[:], Act.Exp, scale=-1.0)
                l1t = l1p.tile([P, F], f32, tag="l1")
                nc.scalar.activation(
                    l1t[:], ut[:], Act.Ln, bias=1.0,
                    accum_out=accL1[:, k : k + 1],
                )
                l5t = l5p.tile([P, F], f32, tag="l5")
                nc.scalar.activation(
                    l5t[:], ut[:], Act.Ln, bias=bclip[:], scale=1.0 + CLIP
                )
                # m = min(L05, L1)
                nc.vector.tensor_tensor(l5t[:], l5t[:], l1t[:], Alu.min)
                # accM[:,k] = sum((y-1)*m); ut is dead, reuse as the dense out
                nc.vector.scalar_tensor_tensor(
                    ut[:], yt[:], 1.0, l5t[:], Alu.subtract, Alu.mult,
                    accum_out=accM[:, k : k + 1],
                )

            # ---------------- correction stream: sum(active * (-loss)) -----
            mrt = maskp.tile([P, U_PAD], f32, tag="mrt")
            nc.sync.dma_start(mrt[:], mr[:])
            mdt = maskp.tile([P, U_PAD], f32, tag="mdt")
            nc.sync.dma_start(mdt[:], md[:])
            mct = maskp.tile([P, U_PAD], f32, tag="mct")
            nc.sync.dma_start(mct[:], mc[:])

            for bt in range(NBT):
                rs = slice(bt * P, (bt + 1) * P)
                xgt = gp.tile([P, U_PAD], f32, tag="xg")
                nc.sync.dma_start(xgt[:], xg[rs, :])
                ygt = gp.tile([P, U_PAD], f32, tag="yg")
                nc.sync.dma_start(ygt[:], yg[rs, :])

                ugt = gp.tile([P, U_PAD], f32, tag="ug")
                nc.scalar.activation(ugt[:], xgt[:], Act.Exp, scale=-1.0)
                l1g = gp.tile([P, U_PAD], f32, tag="l1g")
                nc.scalar.activation(l1g[:], ugt[:], Act.Ln, bias=1.0)
                l5g = gp.tile([P, U_PAD], f32, tag="l5g")
                nc.scalar.activation(
                    l5g[:], ugt[:], Act.Ln, bias=bclip[:], scale=1.0 + CLIP
                )
                # m
                nc.vector.tensor_tensor(l5g[:], l5g[:], l1g[:], Alu.min)
                # n1 = (y-1)*m
                nc.vector.scalar_tensor_tensor(
                    l5g[:], ygt[:], 1.0, l5g[:], Alu.subtract, Alu.mult
                )
                # lg_neg = -loss = L1 + n1
                nc.vector.tensor_tensor(l1g[:], l1g[:], l5g[:], Alu.add)

                # has-group row sums: sum(y * mask)
                hrs = flagp.tile([P, 1], f32, tag="hrs")
                jr = junkp.tile([P, U_PAD], f32, tag="junk")
                nc.vector.scalar_tensor_tensor(
                    jr[:], ygt[:], 0.0, mrt[:], Alu.bypass, Alu.mult,
                    accum_out=hrs[:],
                )
                hds = flagp.tile([P, 1], f32, tag="hds")
                jd = junkp.tile([P, U_PAD], f32, tag="junk")
                nc.vector.scalar_tensor_tensor(
                    jd[:], ygt[:], 0.0, mdt[:], Alu.bypass, Alu.mult,
                    accum_out=hds[:],
                )
                hcs = flagp.tile([P, 1], f32, tag="hcs")
                jc = junkp.tile([P, U_PAD], f32, tag="junk")
                nc.vector.scalar_tensor_tensor(
                    jc[:], ygt[:], 0.0, mct[:], Alu.bypass, Alu.mult,
                    accum_out=hcs[:],
                )

                # any = (hrs+hds+hcs) > 0 ; flags hr/hd/hc = (h*s > 0)
                anys = flagp.tile([P, 1], f32, tag="anys")
                nc.vector.tensor_tensor(anys[:], hrs[:], hds[:], Alu.add)
                nc.vector.tensor_tensor(anys[:], anys[:], hcs[:], Alu.add)
                anyb = flagp.tile([P, 1], f32, tag="anyb")
                nc.vector.tensor_scalar(anyb[:], anys[:], 0.0, None, Alu.is_gt)
                hrb = flagp.tile([P, 1], f32, tag="hrb")
                nc.vector.tensor_scalar(hrb[:], hrs[:], 0.0, None, Alu.is_gt)
                hdb = flagp.tile([P, 1], f32, tag="hdb")
                nc.vector.tensor_scalar(hdb[:], hds[:], 0.0, None, Alu.is_gt)
                hcb = flagp.tile([P, 1], f32, tag="hcb")
                nc.vector.tensor_scalar(hcb[:], hcs[:], 0.0, None, Alu.is_gt)

                # a_g = any & ~has_g  (0/1 floats: any - has_g)
                ar = flagp.tile([P, 1], f32, tag="ar")
                nc.vector.tensor_tensor(ar[:], anyb[:], hrb[:], Alu.subtract)
                ad = flagp.tile([P, 1], f32, tag="ad")
                nc.vector.tensor_tensor(ad[:], anyb[:], hdb[:], Alu.subtract)
                ac = flagp.tile([P, 1], f32, tag="ac")
                nc.vector.tensor_tensor(ac[:], anyb[:], hcb[:], Alu.subtract)

                # active = min(ar*mr + ad*md + ac*mc, 1)
                av = gp.tile([P, U_PAD], f32, tag="av")
                nc.vector.tensor_scalar(av[:], mrt[:], ar[:], None, Alu.mult)
                nc.vector.scalar_tensor_tensor(
                    av[:], mdt[:], ad[:], av[:], Alu.mult, Alu.add
                )
                nc.vector.scalar_tensor_tensor(
                    av[:], mct[:], ac[:], av[:], Alu.mult, Alu.add
                )
                nc.vector.tensor_scalar(av[:], av[:], 1.0, None, Alu.min)

                # accC[:,bt] = sum(active * lg_neg)
                ja = junkp.tile([P, U_PAD], f32, tag="junk")
                nc.vector.scalar_tensor_tensor(
                    ja[:], av[:], 0.0, l1g[:], Alu.bypass, Alu.mult,
                    accum_out=accC[:, bt : bt + 1],
                )

            # ---------------- final combine -> out [P, 2] -------------------
            mainr = finp.tile([P, 1], f32, tag="mainr")
            nc.vector.tensor_reduce(mainr[:], accL1[:], AX.X, Alu.add)
            tmpr = finp.tile([P, 1], f32, tag="tmpr")
            nc.vector.tensor_reduce(tmpr[:], accM[:], AX.X, Alu.add)
            nc.vector.tensor_tensor(mainr[:], mainr[:], tmpr[:], Alu.add)
            corrr = finp.tile([P, 1], f32, tag="corrr")
            nc.vector.tensor_reduce(corrr[:], accC[:], AX.X, Alu.add)

            osb = finp.tile([P, 2], f32, tag="osb")
            nc.vector.tensor_copy(out=osb[:, 0:1], in_=mainr[:])
            nc.vector.tensor_copy(out=osb[:, 1:2], in_=corrr[:])
            nc.sync.dma_start(out[:], osb[:])

    nc.compile()
    return nc


def _get_nc():
    global _NC
    if _NC is None:
        _NC = _build_program()
    return _NC


def _ensure_ntff_hook():
    """Register the axon NTFF profile hook if the image's antenv lacks it."""
    import contextlib
    import ctypes
    import types

    try:
        from antenv.axon_hooks import get_axon_ntff_profile_hook  # noqa: F401
        return
    except ImportError:
        pass

    so_path = "/opt/axon/libaxon_pjrt.so"
    try:
        lib = ctypes.CDLL(so_path)
    except OSError:
        return
    if not hasattr(lib, "axon_start_nrt_profile"):
        return
    lib.axon_start_nrt_profile.argtypes = [
        ctypes.POINTER(ctypes.c_int64),
        ctypes.c_size_t,
    ]
    lib.axon_start_nrt_profile.restype = ctypes.c_int64
    lib.axon_stop_nrt_profile.argtypes = [ctypes.c_char_p]
    lib.axon_stop_nrt_profile.restype = ctypes.c_int64

    @contextlib.contextmanager
    def _hook(output_dir, device_ids):
        import jax

        jax.devices()
        if device_ids:
            ids = (ctypes.c_int64 * len(device_ids))(*device_ids)
            rc = lib.axon_start_nrt_profile(ids, len(device_ids))
        else:
            rc = lib.axon_start_nrt_profile(None, 0)
        if rc != 0:
            raise RuntimeError(f"axon_start_nrt_profile rc={rc}")
        try:
            yield
        finally:
            n = lib.axon_stop_nrt_profile(str(output_dir).encode())
            print(f"ntff profile: {n} file(s) written to {output_dir}",
                  file=sys.stderr)

    mod = types.ModuleType("antenv.axon_hooks")
    mod.get_axon_ntff_profile_hook = lambda: _hook
    mod.set_axon_ntff_profile_hook = lambda h: None
    sys.modules["antenv.axon_hooks"] = mod


def kernel(x, y, recycle_ind, donate_ind, compost_ind):
    global LAST_RESULTS
    import concourse.bass_utils as bass_utils

    # Avoid any network artifact upload in the (optional) trace path.
    bass_utils.upload_artifacts = lambda tmpdir: "local://" + tmpdir
    _ensure_ntff_hook()

    x = np.ascontiguousarray(x, dtype=np.float32)
    y = np.ascontiguousarray(y, dtype=np.float32)
    recycle_ind = np.asarray(recycle_ind).astype(np.int64)
    donate_ind = np.asarray(donate_ind).astype(np.int64)
    compost_ind = np.asarray(compost_ind).astype(np.int64)

    # Union of group columns, padded to the fixed program width. Pad
    # columns carry zero masks so `active` (and thus the correction) is 0.
    cols = np.unique(np.concatenate([recycle_ind, donate_ind, compost_ind]))
    u = len(cols)
    assert u <= U_PAD, (u, U_PAD)
    colsp = np.concatenate([cols, np.zeros(U_PAD - u, dtype=cols.dtype)])

    def mask_b(ind):
        v = np.zeros(U_PAD, np.float32)
        v[:u] = np.isin(cols, ind).astype(np.float32)
        return np.ascontiguousarray(np.broadcast_to(v, (P, U_PAD)))

    mrb = mask_b(recycle_ind)
    mdb = mask_b(donate_ind)
    mcb = mask_b(compost_ind)

    xg = np.ascontiguousarray(x[:, colsp])
    yg = np.ascontiguousarray(y[:, colsp])

    nc = _get_nc()

    in_maps = []
    for i in range(NCORES):
        rs = slice(i * ROWS, (i + 1) * ROWS)
        in_maps.append({
            "x": x[rs].reshape(P, FREE),
            "y": y[rs].reshape(P, FREE),
            "xg": xg[rs],
            "yg": yg[rs],
            "mr": mrb,
            "md": mdb,
            "mc": mcb,
        })

    res = bass_utils.run_bass_kernel_spmd(
        nc, in_maps, core_ids=list(range(NCORES)), trace=TRACE
    )
    LAST_RESULTS = res

    main_neg = 0.0
    corr_neg = 0.0
    for r in res.results:
        o = r["out"].astype(np.float64)
        main_neg += o[:, 0].sum()
        corr_neg += o[:, 1].sum()

    total = main_neg - (1.0 - ALPHA) * corr_neg
    return np.asarray(total, dtype=np.float32)


# revision 6
# speedup vs baseline: 1.3878x; 1.3878x over previous
"""Trainium2 Bass kernel for nn_AsymmetricLossCustom (8 NeuronCores).

Math (reference):
    s  = sigmoid(x)
    t  = min(1 - s + 0.05, 1)
    loss = y*ln(max(s,eps)) + (1-y)*ln(max(t,eps))        # [B, C]
    active[b,c] = OR_g ( (any_g[b] & ~has_g[b]) & mask_g[c] )
    out = -(loss * where(active, 0.1, 1.0)).sum()

Device scheme (2 ScalarE passes + 3 VectorE passes per element):
    sp = sigmoid(-x)                     # ACT (sigmoid table set)
    t  = min(sp + 0.05, 1)               # DVE tensor_scalar dual-op, 2x mode
    c  = 1 - sp            ( = s )       # DVE tensor_scalar dual-op, 2x mode
    w  = y ? c : t                       # DVE copy_predicated
    loss = ln(w)                         # ACT (natural_log set), fused
                                         #   accum_out => per-row sum(loss)

Since sigmoid and ln live in different ACT table sets, chunks are processed
in groups: all sigmoids of a group, then all lns — 2 table loads per group
instead of 2 per chunk.

The `active` down-weighting only touches columns appearing in one of the
three index arrays (<=170 of 9605), so it is handled as a correction term
over host-gathered columns:
    out = -sum(loss) + 0.9 * sum(loss * active) = -sum(loss) - 0.9 * corr_neg
with corr_neg = sum(active * (-loss)) accumulated on device.

Sharding: pure data parallel over the batch. Each core gets 512 rows,
viewed as [128 partitions, 38420 free] (4 rows per partition, contiguous),
plus gathered columns [512, 176] and replicated [128, 176] group masks.
Host sums the 8 per-core [128, 2] partial outputs:
    result = -sum(out[:,0]) - 0.9 * sum(out[:,1])
"""

import sys

import numpy as np

if "/opt/trn_rl_repo" not in sys.path:
    sys.path.insert(0, "/opt/trn_rl_repo")

B, C = 4096, 9605
NCORES = 8
ROWS = B // NCORES          # 512 rows per core
P = 128                     # SBUF partitions
FREE = (ROWS // P) * C      # 38420 f32 per partition
NCHUNK = 17
F = FREE // NCHUNK          # 2260
GROUPS = [range(0, 6), range(6, 12), range(12, 17)]
NBT = ROWS // P             # 4 gathered batch-tiles per core
U_PAD = 176                 # padded union-column count (>= 70+70+30)
CLIP = 0.05
ALPHA = 0.1

TRACE = False               # set True (e.g. from test.py) to capture an NTFF profile
LAST_RESULTS = None         # BassKernelResults of the most recent run

_NC = None


def _build_program():
    import concourse.bacc as bacc
    import concourse.mybir as mybir
    from concourse import tile

    f32 = mybir.dt.float32
    Alu = mybir.AluOpType
    Act = mybir.ActivationFunctionType
    AX = mybir.AxisListType

    nc = bacc.Bacc(
        "TRN2",
        target_bir_lowering=False,
        debug=False,
        enable_asserts=False,
        num_devices=NCORES,
    )

    x = nc.dram_tensor("x", [P, FREE], f32, kind="ExternalInput").ap()
    y = nc.dram_tensor("y", [P, FREE], f32, kind="ExternalInput").ap()
    xg = nc.dram_tensor("xg", [ROWS, U_PAD], f32, kind="ExternalInput").ap()
    yg = nc.dram_tensor("yg", [ROWS, U_PAD], f32, kind="ExternalInput").ap()
    mr = nc.dram_tensor("mr", [P, U_PAD], f32, kind="ExternalInput").ap()
    md = nc.dram_tensor("md", [P, U_PAD], f32, kind="ExternalInput").ap()
    mc = nc.dram_tensor("mc", [P, U_PAD], f32, kind="ExternalInput").ap()
    out = nc.dram_tensor("out", [P, 2], f32, kind="ExternalOutput").ap()

    with tile.TileContext(nc) as tc:
        with (
            tc.tile_pool(name="xp", bufs=3) as xp,
            tc.tile_pool(name="yp", bufs=3) as yp,
            tc.tile_pool(name="sp", bufs=2) as sp,
            tc.tile_pool(name="wp", bufs=7) as wp,
            tc.tile_pool(name="cp", bufs=3) as cp,
            tc.tile_pool(name="accp", bufs=1) as accp,
            tc.tile_pool(name="gp", bufs=5) as gp,
            tc.tile_pool(name="maskp", bufs=1) as maskp,
            tc.tile_pool(name="junkp", bufs=3) as junkp,
            tc.tile_pool(name="flagp", bufs=5) as flagp,
            tc.tile_pool(name="finp", bufs=1) as finp,
        ):
            accLW = accp.tile([P, NCHUNK], f32, tag="accLW")
            accC = accp.tile([P, NBT], f32, tag="accC")

            # replicated group masks (loaded once)
            mrt = maskp.tile([P, U_PAD], f32, tag="mrt")
            nc.sync.dma_start(mrt[:], mr[:])
            mdt = maskp.tile([P, U_PAD], f32, tag="mdt")
            nc.sync.dma_start(mdt[:], md[:])
            mct = maskp.tile([P, U_PAD], f32, tag="mct")
            nc.sync.dma_start(mct[:], mc[:])

            # gathered-column tiles (4 batch tiles, all in group 0's phases)
            g_x, g_y, g_s, g_w, g_av = [], [], [], [], []

            for gi, grp in enumerate(GROUPS):
                # ---- DMA + sigmoid phase -------------------------------
                xts, yts, sts = {}, {}, {}
                for k in grp:
                    cs = slice(k * F, (k + 1) * F)
                    xt = xp.tile([P, F], f32, tag="x")
                    nc.sync.dma_start(xt[:], x[:, cs])
                    yt = yp.tile([P, F], f32, tag="y")
                    nc.sync.dma_start(yt[:], y[:, cs])
                    st = sp.tile([P, F], f32, tag="s")
                    nc.scalar.activation(st[:], xt[:], Act.Sigmoid, scale=-1.0)
                    xts[k], yts[k], sts[k] = xt, yt, st
                if gi == 0:
                    for bt in range(NBT):
                        rs = slice(bt * P, (bt + 1) * P)
                        xgt = gp.tile([P, U_PAD], f32, tag="xg")
                        nc.sync.dma_start(xgt[:], xg[rs, :])
                        ygt = gp.tile([P, U_PAD], f32, tag="yg")
                        nc.sync.dma_start(ygt[:], yg[rs, :])
                        sgt = gp.tile([P, U_PAD], f32, tag="sg")
                        nc.scalar.activation(sgt[:], xgt[:], Act.Sigmoid,
                                             scale=-1.0)
                        g_x.append(xgt)
                        g_y.append(ygt)
                        g_s.append(sgt)

                # ---- DVE blend phase -----------------------------------
                wts = {}
                for k in grp:
                    st, yt = sts[k], yts[k]
                    wt = wp.tile([P, F], f32, tag="w")
                    nc.vector.tensor_scalar(wt[:], st[:], CLIP, 1.0,
                                            Alu.add, Alu.min)
                    ct = cp.tile([P, F], f32, tag="c")
                    nc.vector.tensor_scalar(ct[:], st[:], -1.0, 1.0,
                                            Alu.mult, Alu.add)
                    nc.vector.copy_predicated(
                        wt[:], yt[:].bitcast(mybir.dt.int32), ct[:])
                    wts[k] = wt
                if gi == 0:
                    for bt in range(NBT):
                        sgt, ygt = g_s[bt], g_y[bt]
                        wgt = gp.tile([P, U_PAD], f32, tag="wg")
                        nc.vector.tensor_scalar(wgt[:], sgt[:], CLIP, 1.0,
                                                Alu.add, Alu.min)
                        cgt = gp.tile([P, U_PAD], f32, tag="cg")
                        nc.vector.tensor_scalar(cgt[:], sgt[:], -1.0, 1.0,
                                                Alu.mult, Alu.add)
                        nc.vector.copy_predicated(
                            wgt[:], ygt[:].bitcast(mybir.dt.int32), cgt[:])
                        g_w.append(wgt)

                        # has-group row sums: sum(y * mask) -> flags
                        hrs = flagp.tile([P, 1], f32, tag="hrs")
                        jr = junkp.tile([P, U_PAD], f32, tag="junk")
                        nc.vector.scalar_tensor_tensor(
                            jr[:], ygt[:], 0.0, mrt[:], Alu.bypass, Alu.mult,
                            accum_out=hrs[:],
                        )
                        hds = flagp.tile([P, 1], f32, tag="hds")
                        jd = junkp.tile([P, U_PAD], f32, tag="junk")
                        nc.vector.scalar_tensor_tensor(
                            jd[:], ygt[:], 0.0, mdt[:], Alu.bypass, Alu.mult,
                            accum_out=hds[:],
                        )
                        hcs = flagp.tile([P, 1], f32, tag="hcs")
                        jc = junkp.tile([P, U_PAD], f32, tag="junk")
                        nc.vector.scalar_tensor_tensor(
                            jc[:], ygt[:], 0.0, mct[:], Alu.bypass, Alu.mult,
                            accum_out=hcs[:],
                        )

                        anys = flagp.tile([P, 1], f32, tag="anys")
                        nc.vector.tensor_tensor(anys[:], hrs[:], hds[:], Alu.add)
                        nc.vector.tensor_tensor(anys[:], anys[:], hcs[:], Alu.add)
                        anyb = flagp.tile([P, 1], f32, tag="anyb")
                        nc.vector.tensor_scalar(anyb[:], anys[:], 0.0, None,
                                                Alu.is_gt)
                        hrb = flagp.tile([P, 1], f32, tag="hrb")
                        nc.vector.tensor_scalar(hrb[:], hrs[:], 0.0, None,
                                                Alu.is_gt)
                        hdb = flagp.tile([P, 1], f32, tag="hdb")
                        nc.vector.tensor_scalar(hdb[:], hds[:], 0.0, None,
                                                Alu.is_gt)
                        hcb = flagp.tile([P, 1], f32, tag="hcb")
                        nc.vector.tensor_scalar(hcb[:], hcs[:], 0.0, None,
                                                Alu.is_gt)

                        # a_g = any - has_g   (0/1 floats)
                        ar = flagp.tile([P, 1], f32, tag="ar")
                        nc.vector.tensor_tensor(ar[:], anyb[:], hrb[:],
                                                Alu.subtract)
                        ad = flagp.tile([P, 1], f32, tag="ad")
                        nc.vector.tensor_tensor(ad[:], anyb[:], hdb[:],
                                                Alu.subtract)
                        ac = flagp.tile([P, 1], f32, tag="ac")
                        nc.vector.tensor_tensor(ac[:], anyb[:], hcb[:],
                                                Alu.subtract)

                        # active = min(ar*mr + ad*md + ac*mc, 1)
                        av = gp.tile([P, U_PAD], f32, tag="av")
                        nc.vector.tensor_scalar(av[:], mrt[:], ar[:], None,
                                                Alu.mult)
                        nc.vector.scalar_tensor_tensor(
                            av[:], mdt[:], ad[:], av[:], Alu.mult, Alu.add)
                        nc.vector.scalar_tensor_tensor(
                            av[:], mct[:], ac[:], av[:], Alu.mult, Alu.add)
                        nc.vector.tensor_scalar(av[:], av[:], 1.0, None,
                                                Alu.min)
                        g_av.append(av)

                # ---- Ln phase ------------------------------------------
                for k in grp:
                    wt = wts[k]
                    lt = cp.tile([P, F], f32, tag="c")
                    nc.scalar.activation(lt[:], wt[:], Act.Ln,
                                         accum_out=accLW[:, k : k + 1])
                if gi == 0:
                    for bt in range(NBT):
                        lgt = gp.tile([P, U_PAD], f32, tag="lg")
                        nc.scalar.activation(lgt[:], g_w[bt][:], Act.Ln)
                        # accC[:,bt] = sum(-active * loss)
                        ja = junkp.tile([P, U_PAD], f32, tag="junk")
                        nc.vector.scalar_tensor_tensor(
                            ja[:], g_av[bt][:], -1.0, lgt[:],
                            Alu.mult, Alu.mult,
                            accum_out=accC[:, bt : bt + 1],
                        )

            # ---- final combine -> out [P, 2] ---------------------------
            lossr = finp.tile([P, 1], f32, tag="lossr")
            nc.vector.tensor_reduce(lossr[:], accLW[:], AX.X, Alu.add)
            corrr = finp.tile([P, 1], f32, tag="corrr")
            nc.vector.tensor_reduce(corrr[:], accC[:], AX.X, Alu.add)

            osb = finp.tile([P, 2], f32, tag="osb")
            nc.vector.tensor_copy(out=osb[:, 0:1], in_=lossr[:])
            nc.vector.tensor_copy(out=osb[:, 1:2], in_=corrr[:])
            nc.sync.dma_start(out[:], osb[:])

    nc.compile()
    return nc


def _get_nc():
    global _NC
    if _NC is None:
        _NC = _build_program()
    return _NC


def _ensure_ntff_hook():
    """Register the axon NTFF profile hook if the image's antenv lacks it."""
    import contextlib
    import ctypes
    import types

    try:
        from antenv.axon_hooks import get_axon_ntff_profile_hook  # noqa: F401
        return
    except ImportError:
        pass

    so_path = "/opt/axon/libaxon_pjrt.so"
    try:
        lib = ctypes.CDLL(so_path)
    except OSError:
        return
    if not hasattr(lib, "axon_start_nrt_profile"):
        return
    lib.axon_start_nrt_profile.argtypes = [
        ctypes.POINTER(ctypes.c_int64),
        ctypes.c_size_t,
    ]
    lib.axon_start_nrt_profile.restype = ctypes.c_int64
    lib.axon_stop_nrt_profile.argtypes = [ctypes.c_char_p]
    lib.axon_stop_nrt_profile.restype = ctypes.c_int64

    @contextlib.contextmanager
    def _hook(output_dir, device_ids):
        import jax

        jax.devices()
        if device_ids:
            ids = (ctypes.c_int64 * len(device_ids))(*device_ids)
            rc = lib.axon_start_nrt_profile(ids, len(device_ids))
        else:
            rc = lib.axon_start_nrt_profile(None, 0)
        if rc != 0:
            raise RuntimeError(f"axon_start_nrt_profile rc={rc}")
        try:
            yield
        finally:
            n = lib.axon_stop_nrt_profile(str(output_dir).encode())
            print(f"ntff profile: {n} file(s) written to {output_dir}",
                  file=sys.stderr)

    mod = types.ModuleType("antenv.axon_hooks")
    mod.get_axon_ntff_profile_hook = lambda: _hook
    mod.set_axon_ntff_profile_hook = lambda h: None
    sys.modules["antenv.axon_hooks"] = mod


def kernel(x, y, recycle_ind, donate_ind, compost_ind):
    global LAST_RESULTS
    import concourse.bass_utils as bass_utils

    # Avoid any network artifact upload in the (optional) trace path.
    bass_utils.upload_artifacts = lambda tmpdir: "local://" + tmpdir
    _ensure_ntff_hook()

    x = np.ascontiguousarray(x, dtype=np.float32)
    y = np.ascontiguousarray(y, dtype=np.float32)
    recycle_ind = np.asarray(recycle_ind).astype(np.int64)
    donate_ind = np.asarray(donate_ind).astype(np.int64)
    compost_ind = np.asarray(compost_ind).astype(np.int64)

    # Union of group columns, padded to the fixed program width. Pad
    # columns carry zero masks so `active` (and thus the correction) is 0.
    cols = np.unique(np.concatenate([recycle_ind, donate_ind, compost_ind]))
    u = len(cols)
    assert u <= U_PAD, (u, U_PAD)
    colsp = np.concatenate([cols, np.zeros(U_PAD - u, dtype=cols.dtype)])

    def mask_b(ind):
        v = np.zeros(U_PAD, np.float32)
        v[:u] = np.isin(cols, ind).astype(np.float32)
        return np.ascontiguousarray(np.broadcast_to(v, (P, U_PAD)))

    mrb = mask_b(recycle_ind)
    mdb = mask_b(donate_ind)
    mcb = mask_b(compost_ind)

    xg = np.ascontiguousarray(x[:, colsp])
    yg = np.ascontiguousarray(y[:, colsp])

    nc = _get_nc()

    in_maps = []
    for i in range(NCORES):
        rs = slice(i * ROWS, (i + 1) * ROWS)
        in_maps.append({
            "x": x[rs].reshape(P, FREE),
            "y": y[rs].reshape(P, FREE),
            "xg": xg[rs],
            "yg": yg[rs],
            "mr": mrb,
            "md": mdb,
            "mc": mcb,
        })

    res = bass_utils.run_bass_kernel_spmd(
        nc, in_maps, core_ids=list(range(NCORES)), trace=TRACE
    )
    LAST_RESULTS = res

    loss_sum = 0.0
    corr_neg = 0.0
    for r in res.results:
        o = r["out"].astype(np.float64)
        loss_sum += o[:, 0].sum()
        corr_neg += o[:, 1].sum()

    total = -loss_sum - (1.0 - ALPHA) * corr_neg
    return np.asarray(total, dtype=np.float32)
